# revision 29
# baseline (speedup 1.0000x reference)
"""Trainium2 Bass kernel for nn_MemoryNetwork (scatter_memory).

Computation (reference, per batch row b):
    f = feature / ||feature||                       [B, 768]
    topic = f @ W_topic.T ; dom = f @ W_domain.T    [B, 256]
    att   = softmax_m(TAU * topic . memory[d,m])    [B, 9, 10]
    sep   = sum_m att * memory[d,m]                 [B, 9, 256]
    out   = softmax_d(TAU * sep . dom)              [B, 1, 9]

Reformulation: fold the tiny memory banks into the projections on host:
    R = [mem_flat @ W_topic; mem_flat @ W_domain]   [180, 768]
and pre-scale rows by r = TAU/||f|| (host): fs = r*f. Then per row
    raw  = fs @ R.T          (rawS = raw[:90], rawT = raw[90:])
    ex   = exp(rawS - SHIFT) (const shift; logits in [-130, 110])
    datt = (sum_m ex*rawT) / (sum_m ex)   per domain
    out  = softmax_d(datt)   (const shift again)

Precision: fs and R are split fp16 + residual; residual corrections are
applied only to the att-logit half (columns 0:90) — the dom half enters
the output linearly and tolerates fp16-level error:
    raw        = fhi16 @ Rhi16          (6 fp16 matmuls, N=180)
    raw[0:90] += fhi16 @ Rlo16[0:90]    (6 fp16 matmuls, N=90)
    raw[0:90] += flo8  @ Rhi8[0:90]     (3 fp8 e5m2 DoubleRow matmuls,
                                         2 k-chunks each, N=90)
Measured end-to-end error ~6.9e-3 vs the fp32 reference (gate 2e-2).
(Tried and rejected: fp8 copy of fhi for a DoubleRow Rlo pass — every
cast route (DMA-engine cast, gpsimd DSP cast, ACT/DVE copies) costs more
than it saves; fusing Rlo into one N=270 stream — the wider stream makes
the following DoubleRow instructions run at the wide-issue rate.)

Sharding: data-parallel over B across 8 cores (4096 rows each). Features
are shipped pre-tiled [128, NT, KC, 128] flattened so each DMA block is
one contiguous run per partition (hi fp16 + lo fp8e5m2 = 3 B/elem,
9.4 MB/core). hi + out ride the sync queue, lo + fp8/lo R the scalar
queue, hi-R first on sync so matmul 0 starts as early as possible.

Schedule: half-groups of 4 batch tiles accumulate into one 4-bank PSUM
tile (ping-pong, 8 banks); exp/copy epilogue runs as batched ACT ops
across the 4 banks; the softmax tail runs as grouped DVE chains with
staggered shrinking groups (8,8,8,4,2,2) so the serial chain after the
last matmul is a 2-tile group whose predecessor overlapped the final
matmuls.
"""

import sys

sys.path.insert(0, "/opt/trn_rl_repo")

import numpy as np
import ml_dtypes

B, IN, E, D, M = 32768, 768, 256, 9, 10
NCORES = 8
BC = B // NCORES   # rows per core
P = 128            # partition tile
NT = BC // P       # batch tiles per core (32)
KC = IN // P       # contraction chunks (6)
DM = 2 * D * M     # 180
TAU = 32.0
SHIFT = 50.0
H = NT // 4        # half-groups of 4 tiles (8)
BLOCKS = [1, 1, 2, 2, 2] + [4] * 6     # feature DMA block sizes (tiles)
EPI_GROUPS = [(0, 8), (8, 8), (16, 8), (24, 4), (28, 2), (30, 2)]

_CACHE: dict = {}


def _build_nc(repeat=1):
    from contextlib import ExitStack

    import concourse.bacc as bacc
    import concourse.tile as tile
    from concourse import mybir

    F32 = mybir.dt.float32
    F16 = mybir.dt.float16
    E5 = mybir.dt.float8e5
    AF = mybir.ActivationFunctionType
    DR = mybir.MatmulPerfMode.DoubleRow
    AX = mybir.AxisListType.X

    TW = KC * P  # elems per (partition, tile) row = 768
    nc = bacc.Bacc(trn_type="TRN2")
    fhi = nc.dram_tensor("fhi", [P, NT * TW], F16, kind="ExternalInput")
    flo = nc.dram_tensor("flo", [P, NT * TW], E5, kind="ExternalInput")
    rthi = nc.dram_tensor("rthi", [P, KC * DM], F16, kind="ExternalInput")
    rtlo = nc.dram_tensor("rtlo", [P, KC * D * M], F16, kind="ExternalInput")
    rt8 = nc.dram_tensor("rt8", [P, KC * D * M], E5, kind="ExternalInput")
    out = nc.dram_tensor("out", [BC, D], F32, kind="ExternalOutput")

    with tile.TileContext(nc) as tc, ExitStack() as ctx:
        const = ctx.enter_context(tc.tile_pool(name="const", bufs=1))
        fpool = ctx.enter_context(tc.tile_pool(name="fts", bufs=7))
        gpool = ctx.enter_context(tc.tile_pool(name="grp", bufs=2))
        spool = ctx.enter_context(tc.tile_pool(name="small", bufs=2))
        raw_ps = ctx.enter_context(tc.tile_pool(name="rawps", bufs=2, space="PSUM"))

        # hi-R on the sync queue first (the scalar engine starts late:
        # act-table + partition-id loads); lo-R and fp8-R on scalar, needed
        # only ~0.5us/0.9us after matmul 0.
        rthi_sb = const.tile([P, KC, DM], F16)
        nc.sync.dma_start(
            rthi_sb[:].rearrange("p k j -> p (k j)"), rthi[:, :])
        rtlo_sb = const.tile([P, KC, D * M], F16)
        nc.scalar.dma_start(
            rtlo_sb[:].rearrange("p k j -> p (k j)"), rtlo[:, :])
        rt8_sb = const.tile([P, KC, D * M], E5)
        nc.scalar.dma_start(
            rt8_sb[:].rearrange("p k j -> p (k j)"), rt8[:, :])
        bias_shift = const.tile([P, 1], F32)
        nc.gpsimd.memset(bias_shift[:], -SHIFT)

        out_v = out[:, :].rearrange("(t p) d -> p t d", p=P)

        for it in range(repeat):
            # Feature DMA blocks: hi on sync, lo on scalar.
            hi_blocks, lo_blocks = [], []
            t0 = 0
            flat = "p b k c -> p (b k c)"
            for bn in BLOCKS:
                hi_sb = fpool.tile([P, bn, KC, P], F16, tag=f"fhi{bn}")
                lo_sb = fpool.tile([P, bn, KC, P], E5, tag=f"flo{bn}")
                nc.sync.dma_start(
                    hi_sb[:].rearrange(flat), fhi[:, t0 * TW : (t0 + bn) * TW])
                nc.scalar.dma_start(
                    lo_sb[:].rearrange(flat), flo[:, t0 * TW : (t0 + bn) * TW])
                hi_blocks.append((t0, bn, hi_sb))
                lo_blocks.append(lo_sb)
                t0 += bn

            def tile_view(blocks, s):
                for i, (b0, bn, _) in enumerate(hi_blocks):
                    if b0 <= s < b0 + bn:
                        return blocks[i][:, s - b0]
                raise AssertionError

            hi_all = [b for _, _, b in hi_blocks]
            raw_halves = []
            for h in range(H):
                raw4 = raw_ps.tile([P, 4, 512], F32, tag="raw4")
                raw_halves.append(raw4)
                for j in range(4):
                    s = 4 * h + j
                    hi_t = tile_view(hi_all, s)
                    lo_t = tile_view(lo_blocks, s)
                    acc = raw4[:, j, 0:DM]
                    for k in range(KC):
                        nc.tensor.matmul(
                            acc, hi_t[:, k, :], rthi_sb[:, k, :],
                            start=(k == 0), stop=False,
                        )
                    for k in range(KC):
                        nc.tensor.matmul(
                            acc[:, 0 : D * M], hi_t[:, k, :], rtlo_sb[:, k, :],
                            start=False, stop=False,
                        )
                    for j2 in range(KC // 2):
                        kk = slice(2 * j2, 2 * j2 + 2)
                        nc.tensor.matmul(
                            acc[:, 0 : D * M], lo_t[:, kk, :], rt8_sb[:, kk, :],
                            start=False, stop=(j2 == KC // 2 - 1), perf_mode=DR,
                        )

            for t0, G in EPI_GROUPS:
                h0, nh = t0 // 4, max(1, G // 4)
                ex_g = gpool.tile([P, G, D * M], F32, tag=f"exg{G}")
                t_g = gpool.tile([P, G, D * M], F32, tag=f"tg{G}")
                for hj in range(nh):
                    raw4 = raw_halves[h0 + hj]
                    j0, jn = (t0 % 4, G) if G < 4 else (0, 4)
                    sl = slice(4 * hj, 4 * hj + jn)
                    nc.scalar.activation(
                        ex_g[:, sl, :], raw4[:, j0 : j0 + jn, 0 : D * M],
                        AF.Exp, bias=bias_shift[:],
                    )
                    nc.scalar.copy(
                        t_g[:, sl, :], raw4[:, j0 : j0 + jn, D * M : DM])

                sums = spool.tile([P, G, D], F32, tag=f"sums{G}")
                nc.vector.reduce_sum(
                    sums[:],
                    ex_g[:].rearrange("p s (d m) -> p s d m", d=D, m=M),
                    axis=AX,
                )
                prod = spool.tile([P, G, D * M], F32, tag=f"prod{G}")
                nc.vector.tensor_mul(prod[:], ex_g[:], t_g[:])
                wsum = spool.tile([P, G, D], F32, tag=f"wsum{G}")
                nc.vector.reduce_sum(
                    wsum[:],
                    prod[:].rearrange("p s (d m) -> p s d m", d=D, m=M),
                    axis=AX,
                )
                rsums = spool.tile([P, G, D], F32, tag=f"rsums{G}")
                nc.vector.reciprocal(rsums[:], sums[:])
                datt = spool.tile([P, G, D], F32, tag=f"datt{G}")
                nc.vector.tensor_mul(datt[:], wsum[:], rsums[:])
                ex2 = spool.tile([P, G, D], F32, tag=f"ex2{G}")
                nc.scalar.activation(ex2[:], datt[:], AF.Exp, bias=bias_shift[:])
                sumd = spool.tile([P, G], F32, tag=f"sumd{G}")
                nc.vector.reduce_sum(sumd[:], ex2[:], axis=AX)
                rd = spool.tile([P, G], F32, tag=f"rd{G}")
                nc.vector.reciprocal(rd[:], sumd[:])
                out_t = spool.tile([P, G, D], F32, tag=f"outt{G}")
                nc.vector.tensor_mul(
                    out_t[:], ex2[:], rd[:, :, None].broadcast_to([P, G, D])
                )
                nc.sync.dma_start(out_v[:, t0 : t0 + G, :], out_t[:])

    # All ACT functions used (Exp, Copy/Identity) live in one table set; steer
    # the table-load placement pass to a single covering set to avoid
    # alternating ~2.7us table loads.
    mine = {AF.Exp, AF.Ln, AF.Square, AF.Copy, AF.Identity}
    orig_tables = bacc.get_activation_tables

    def _patched(arch):
        return {
            name: (fns if name == "natural_log_exp_and_others" else fns - mine)
            for name, fns in orig_tables(arch).items()
        }

    bacc.get_activation_tables = _patched
    try:
        nc.finalize()
    finally:
        bacc.get_activation_tables = orig_tables
    return nc


def _get_nc():
    if "nc" not in _CACHE:
        _CACHE["nc"] = _build_nc()
    return _CACHE["nc"]


def _host_prep(feature, W_topic, W_domain, memory):
    """R splits and per-core pre-scaled, pre-tiled feature splits."""
    E5np = ml_dtypes.float8_e5m2
    mem_flat = memory.reshape(D * M, E).astype(np.float64)
    Pm = mem_flat @ W_topic.astype(np.float64)
    Qm = mem_flat @ W_domain.astype(np.float64)
    RT = np.concatenate([Pm, Qm], axis=0).T.astype(np.float32)  # [768, 180]
    RhiT = RT.astype(np.float16)
    RloT = RT - RhiT.astype(np.float32)
    rhi3 = RhiT.reshape(KC, P, DM)
    rlo3 = RloT.astype(np.float16).reshape(KC, P, DM)
    rthi = np.ascontiguousarray(rhi3.transpose(1, 0, 2)).reshape(P, -1)
    rtlo = np.ascontiguousarray(
        rlo3[:, :, 0 : D * M].transpose(1, 0, 2)).reshape(P, -1)
    rt8 = np.ascontiguousarray(
        rhi3[:, :, 0 : D * M].astype(E5np).transpose(1, 0, 2)).reshape(P, -1)

    f = np.asarray(feature, dtype=np.float64)
    r = TAU / np.sqrt((f ** 2).sum(axis=1))
    fs = (f * r[:, None]).astype(np.float32)

    per_core = []
    for c in range(NCORES):
        ft = fs[c * BC : (c + 1) * BC].T  # [768, BC]
        # [P, NT, KC, P]: tile t, chunk k, col c -> ft[k*128+p, t*128+c]
        tiled = ft.reshape(KC, P, NT, P).transpose(1, 2, 0, 3)
        fhi = np.ascontiguousarray(tiled.astype(np.float16))
        flo = np.ascontiguousarray(
            (tiled - fhi.astype(np.float32)).astype(E5np)).reshape(P, -1)
        fhi = fhi.reshape(P, -1)
        per_core.append(
            {"fhi": fhi, "flo": flo, "rthi": rthi, "rtlo": rtlo, "rt8": rt8})
    return per_core


def kernel(feature, category, W_topic, W_domain, memory):
    from concourse.bass_utils import run_bass_kernel_spmd

    in_maps = _host_prep(
        feature, np.asarray(W_topic), np.asarray(W_domain), np.asarray(memory)
    )
    nc = _get_nc()
    res = run_bass_kernel_spmd(nc, in_maps, core_ids=list(range(NCORES)))
    outs = [res.results[c]["out"] for c in range(NCORES)]
    full = np.concatenate(outs, axis=0)  # [B, 9]
    return full[:, None, :].astype(np.float32)
